# revision 4
# baseline (speedup 1.0000x reference)
"""Trainium2 Bass kernel for nn_DynamicGeometricRotation.

Reference computation (B=16, S=8192, D=128, H=512, R=3):
    pooled = x.mean(S)                           [B, D]
    h      = gelu_exact(pooled @ W1.T + b1)      [B, H]
    params = (h @ W2.T + b2) -> [B, R, D, D]
    for i in 0..R:  g_i = 0.5(p_i - p_i^T);  x = x @ expm(g_i)

Key identity: the rotations depend only on the ORIGINAL x (pooled before the
loop), so out = x @ (R1 @ R2 @ R3) — a single batched einsum.

Device plan (8 cores):
  L1 "pool":   batch-sharded (2 batches/core). Column-sums of x via PE
               (x-tile stationary, ones moving), accumulate in PSUM.
  host:        tiny MLP (pooled @ W1.T + b1, exact-erf gelu) in f64.
  L2 "params": W2 column-sharded (6144 rows of the 49152-row output per
               core, 12 MB instead of 96 MB each). paramsT = W2T_c.T-chunks
               as PE stationary, hT moving.
  host:        add b2, skew-symmetrize -> G (and -G).
  L3 "rot":    batch-sharded. On-device expm via degree-12 Taylor in
               Paterson-Stockmeyer form (5 matmuls; all stationary operands
               are symmetric powers of the skew G, or -G, so no PE
               transposes are needed), rotation chain R1@R2@R3 (2 PE
               transposes), then the einsum out = x @ Rall with per-tile PE
               transposes of x.
"""

import math

import numpy as np

import concourse.bass as bass
import concourse.mybir as mybir
import concourse.tile as tile
from concourse.bass_utils import run_bass_kernel_spmd
from concourse.masks import make_identity

F32 = mybir.dt.float32

B, S, D = 16, 8192, 128
H = 512
NROT = 3
NCORES = 8
BPC = B // NCORES           # batches per core = 2
JPC = NROT * D * D // NCORES  # W2 output rows per core = 6144

_FACT_INV = [1.0 / math.factorial(k) for k in range(13)]


def _split_sync_waits(nc, max_waits=1):
    """walrus in this container rejects >1 semaphore wait per instruction
    ("Too many sync wait commands"). Split extra waits into preceding
    same-engine NOPs (the engine stalls at the NOP, preserving
    happens-before)."""
    for fn in nc.m.functions:
        for bb in fn.blocks:
            insts = bb.instructions
            i = 0
            while i < len(insts):
                inst = insts[i]
                si = inst.sync_info
                if si is not None and len(si.on_wait) > max_waits:
                    waits = list(si.on_wait)
                    keep = waits[-max_waits:]
                    rest = waits[:-max_waits]
                    nops = []
                    for j in range(0, len(rest), max_waits):
                        nops.append(
                            mybir.InstNoOp(
                                name=f"{inst.name}-waitsplit-{j}",
                                engine=inst.engine,
                                sync_info=mybir.SyncInfo(
                                    on_wait=rest[j : j + max_waits], on_update=[]
                                ),
                                bass_nofuse=True,
                            )
                        )
                    inst.sync_info = mybir.SyncInfo(
                        on_wait=keep, on_update=list(si.on_update)
                    )
                    for k, nop in enumerate(nops):
                        insts.insert(i + k, nop)
                    i += len(nops)
                i += 1
    return nc


def build_pool():
    """Per core: x [BPC, S, D] -> pooledT [D, BPC] (sum over S)."""
    nc = bass.Bass(target_bir_lowering=False)
    x = nc.declare_dram_parameter("x", [BPC, S, D], F32, isOutput=False)
    out = nc.declare_dram_parameter("pooledT", [D, BPC], F32, isOutput=True)
    # [b, c, t, p, d]: chunk c = 1024 rows = 8 tiles of 128
    xr = x.rearrange("b (c t p) d -> b c p t d", p=128, t=8)
    nchunk = S // 1024
    with tile.TileContext(nc) as tc:
        with (
            tc.tile_pool(name="xin", bufs=3) as xpool,
            tc.tile_pool(name="one", bufs=1) as spool,
            tc.tile_pool(name="acc", bufs=1, space="PSUM") as pspool,
        ):
            ones = spool.tile([128, 1], F32)
            nc.vector.memset(ones, 1.0)
            acc = pspool.tile([128, BPC], F32)
            res = spool.tile([128, BPC], F32)
            for b in range(BPC):
                for c in range(nchunk):
                    xt = xpool.tile([128, 8, 128], F32)
                    nc.sync.dma_start(out=xt, in_=xr[b, c])
                    for t in range(8):
                        nc.tensor.matmul(
                            acc[:, b : b + 1],
                            lhsT=xt[:, t, :],
                            rhs=ones,
                            start=(c == 0 and t == 0),
                            stop=(c == nchunk - 1 and t == 7),
                        )
            nc.vector.tensor_copy(res, acc)
            nc.sync.dma_start(out=out[:, :], in_=res)
    return _split_sync_waits(nc)


def build_params():
    """Per core: paramsT[j, b] = sum_k W2T_c[k, j] * hT[k, b].
    W2T_c = W2.T[:, c*JPC:(c+1)*JPC]  ([H, JPC], 12 MB)."""
    nc = bass.Bass(target_bir_lowering=False)
    w2t = nc.declare_dram_parameter("w2t", [H, JPC], F32, isOutput=False)
    ht = nc.declare_dram_parameter("ht", [H, B], F32, isOutput=False)
    out = nc.declare_dram_parameter("paramsT", [JPC, B], F32, isOutput=True)
    KT = H // 128          # 4 k-tiles
    JI = 12                # j-tiles per group
    JO = JPC // (JI * 128)  # 4 groups of 1536 columns
    htr = ht.rearrange("(t p) b -> p t b", p=128)
    outr = out.rearrange("(jo ji p) b -> jo p ji b", p=128, ji=JI)
    with tile.TileContext(nc) as tc:
        with (
            tc.tile_pool(name="w", bufs=5) as wpool,
            tc.tile_pool(name="h", bufs=1) as hpool,
            tc.tile_pool(name="o", bufs=2) as opool,
            tc.tile_pool(name="ps", bufs=2, space="PSUM") as pspool,
        ):
            ht_sb = hpool.tile([128, KT, B], F32)
            nc.sync.dma_start(out=ht_sb, in_=htr)
            for jo in range(JO):
                ps = pspool.tile([128, JI, B], F32)
                # load all KT k-chunks for this column group, then accumulate
                # each psum slice k-contiguously (interleaving accumulation
                # groups within one PSUM bank corrupts has_written state).
                ws = []
                for k in range(KT):
                    w = wpool.tile([128, JI * 128], F32, tag="w")
                    nc.sync.dma_start(
                        out=w,
                        in_=w2t[k * 128 : (k + 1) * 128, jo * JI * 128 : (jo + 1) * JI * 128],
                    )
                    ws.append(w)
                for ji in range(JI):
                    for k in range(KT):
                        nc.tensor.matmul(
                            ps[:, ji, :],
                            lhsT=ws[k][:, ji * 128 : (ji + 1) * 128],
                            rhs=ht_sb[:, k, :],
                            start=(k == 0),
                            stop=(k == KT - 1),
                        )
                res = opool.tile([128, JI, B], F32)
                nc.vector.tensor_copy(res, ps)
                nc.sync.dma_start(out=outr[jo], in_=res)
    return _split_sync_waits(nc)


def build_rot():
    """Per core: x [BPC, S, D], g/ng [BPC, NROT, D, D] -> y = x @ expm-chain."""
    nc = bass.Bass(target_bir_lowering=False)
    x = nc.declare_dram_parameter("x", [BPC, S, D], F32, isOutput=False)
    g = nc.declare_dram_parameter("g", [BPC, NROT, D, D], F32, isOutput=False)
    ng = nc.declare_dram_parameter("ng", [BPC, NROT, D, D], F32, isOutput=False)
    y = nc.declare_dram_parameter("y", [BPC, S, D], F32, isOutput=True)

    CH = 512               # einsum chunk rows
    TPC = CH // 128        # tiles per chunk
    nchunk = S // CH
    xr = x.rearrange("b (c t p) d -> b c p t d", p=128, t=TPC)
    yr = y.rearrange("b (c t p) d -> b c p t d", p=128, t=TPC)
    gr = g.rearrange("b r p d -> b p r d")
    ngr = ng.rearrange("b r p d -> b p r d")

    C = _FACT_INV

    with tile.TileContext(nc) as tc:
        with (
            tc.tile_pool(name="const", bufs=1) as cpool,
            tc.tile_pool(name="gin", bufs=2) as gpool,
            tc.tile_pool(name="expm", bufs=2) as epool,
            tc.tile_pool(name="rmat", bufs=2) as rpool,
            tc.tile_pool(name="xin", bufs=3) as xpool,
            tc.tile_pool(name="xt", bufs=3) as xtpool,
            tc.tile_pool(name="yout", bufs=3) as ypool,
            tc.tile_pool(name="psA", bufs=2, space="PSUM") as psA,
            tc.tile_pool(name="psB", bufs=2, space="PSUM") as psB,
            tc.tile_pool(name="psE", bufs=2, space="PSUM") as psE,
        ):
            ident = cpool.tile([128, 128], F32)
            make_identity(nc, ident)

            rall_tiles = []
            for b in range(BPC):
                g_sb = gpool.tile([128, NROT, 128], F32, tag="g_sb")
                ng_sb = gpool.tile([128, NROT, 128], F32, tag="ng_sb")
                nc.sync.dma_start(out=g_sb, in_=gr[b])
                nc.sync.dma_start(out=ng_sb, in_=ngr[b])

                r_sb = []
                for i in range(NROT):
                    gi = g_sb[:, i, :]
                    ngi = ng_sb[:, i, :]
                    # powers: G2 = G@G (lhsT=-G), G3 = G2@G, G4 = G2@G2
                    g2p = psE.tile([128, 128], F32, tag="ep")
                    nc.tensor.matmul(g2p, lhsT=ngi, rhs=gi, start=True, stop=True)
                    g2 = epool.tile([128, 128], F32, tag="g2")
                    nc.vector.tensor_copy(g2, g2p)
                    g3p = psE.tile([128, 128], F32, tag="ep")
                    nc.tensor.matmul(g3p, lhsT=g2, rhs=gi, start=True, stop=True)
                    g3 = epool.tile([128, 128], F32, tag="g3")
                    nc.vector.tensor_copy(g3, g3p)
                    g4p = psE.tile([128, 128], F32, tag="ep")
                    nc.tensor.matmul(g4p, lhsT=g2, rhs=g2, start=True, stop=True)
                    g4 = epool.tile([128, 128], F32, tag="g4")
                    nc.vector.tensor_copy(g4, g4p)

                    A = mybir.AluOpType
                    # inner = c12*G4 + (c8 I + c9 G + c10 G2 + c11 G3)
                    t1 = epool.tile([128, 128], F32, tag="t1")
                    nc.vector.tensor_scalar_mul(t1, g3, C[11])
                    nc.vector.scalar_tensor_tensor(t1, g4, C[12], t1, A.mult, A.add)
                    nc.vector.scalar_tensor_tensor(t1, g2, C[10], t1, A.mult, A.add)
                    nc.vector.scalar_tensor_tensor(t1, gi, C[9], t1, A.mult, A.add)
                    nc.vector.scalar_tensor_tensor(t1, ident, C[8], t1, A.mult, A.add)
                    u1p = psE.tile([128, 128], F32, tag="ep")
                    nc.tensor.matmul(u1p, lhsT=g4, rhs=t1, start=True, stop=True)
                    # V = U1 + (c4 I + c5 G + c6 G2 + c7 G3)
                    t2 = epool.tile([128, 128], F32, tag="t2")
                    nc.vector.tensor_scalar_mul(t2, g3, C[7])
                    nc.vector.scalar_tensor_tensor(t2, g2, C[6], t2, A.mult, A.add)
                    nc.vector.scalar_tensor_tensor(t2, gi, C[5], t2, A.mult, A.add)
                    nc.vector.scalar_tensor_tensor(t2, ident, C[4], t2, A.mult, A.add)
                    nc.vector.tensor_tensor(t2, t2, u1p, A.add)
                    u2p = psE.tile([128, 128], F32, tag="ep")
                    nc.tensor.matmul(u2p, lhsT=g4, rhs=t2, start=True, stop=True)
                    # R = U2 + (I + G + G2/2 + G3/6)
                    t3 = epool.tile([128, 128], F32, tag="t3")
                    nc.vector.tensor_scalar_mul(t3, g3, C[3])
                    nc.vector.scalar_tensor_tensor(t3, g2, C[2], t3, A.mult, A.add)
                    nc.vector.tensor_tensor(t3, t3, gi, A.add)
                    nc.vector.tensor_tensor(t3, t3, ident, A.add)
                    ri = rpool.tile([128, 128], F32, tag=f"r{i}")
                    nc.vector.tensor_tensor(ri, t3, u2p, A.add)
                    r_sb.append(ri)

                # chain: Rall = R0 @ R1 @ R2
                t1p = psE.tile([128, 128], F32, tag="ep")
                nc.tensor.transpose(t1p, r_sb[0], ident)
                r0t = epool.tile([128, 128], F32, tag="r0t")
                nc.vector.tensor_copy(r0t, t1p)
                r01p = psE.tile([128, 128], F32, tag="ep")
                nc.tensor.matmul(r01p, lhsT=r0t, rhs=r_sb[1], start=True, stop=True)
                r01 = epool.tile([128, 128], F32, tag="r01")
                nc.vector.tensor_copy(r01, r01p)
                t2p = psE.tile([128, 128], F32, tag="ep")
                nc.tensor.transpose(t2p, r01, ident)
                r01t = epool.tile([128, 128], F32, tag="r01t")
                nc.vector.tensor_copy(r01t, t2p)
                rallp = psE.tile([128, 128], F32, tag="ep")
                nc.tensor.matmul(rallp, lhsT=r01t, rhs=r_sb[2], start=True, stop=True)
                rall = rpool.tile([128, 128], F32, tag="rall")
                nc.vector.tensor_copy(rall, rallp)
                rall_tiles.append(rall)

            for b in range(BPC):
                rall = rall_tiles[b]
                for c in range(nchunk):
                    xt = xpool.tile([128, TPC, 128], F32, tag="xt")
                    nc.sync.dma_start(out=xt, in_=xr[b, c])
                    yt = ypool.tile([128, TPC, 128], F32, tag="yt")
                    for t in range(TPC):
                        xtp = psA.tile([128, 128], F32, tag="xtp")
                        nc.tensor.transpose(xtp, xt[:, t, :], ident)
                        xts = xtpool.tile([128, 128], F32, tag="xts")
                        nc.vector.tensor_copy(xts, xtp)
                        yp = psB.tile([128, 128], F32, tag="yp")
                        nc.tensor.matmul(yp, lhsT=xts, rhs=rall, start=True, stop=True)
                        nc.scalar.copy(yt[:, t, :], yp)
                    nc.sync.dma_start(out=yr[b, c], in_=yt)
    return _split_sync_waits(nc)


_CACHE = {}


def _get(name):
    if name not in _CACHE:
        _CACHE[name] = {"pool": build_pool, "params": build_params, "rot": build_rot}[
            name
        ]()
    return _CACHE[name]


def _erf(z):
    from scipy.special import erf

    return erf(z)


def kernel(x, W1, b1, W2, b2):
    x = np.ascontiguousarray(x, dtype=np.float32)
    cores = list(range(NCORES))

    # ---- L1: pooling ----
    in1 = [{"x": x[c * BPC : (c + 1) * BPC]} for c in cores]
    r1 = run_bass_kernel_spmd(_get("pool"), in1, core_ids=cores)
    pooled = np.concatenate(
        [r1.results[c]["pooledT"].T for c in cores], axis=0
    ).astype(np.float64) / float(S)                     # [B, D]

    # ---- host: tiny MLP with exact-erf gelu ----
    pre = pooled @ W1.astype(np.float64).T + b1.astype(np.float64)
    hh = 0.5 * pre * (1.0 + _erf(pre / np.sqrt(2.0)))
    hT = np.ascontiguousarray(hh.T, dtype=np.float32)   # [H, B]

    # ---- L2: params = h @ W2.T (sharded over W2 rows) ----
    W2T = np.ascontiguousarray(W2.astype(np.float32).T)  # [H, NROT*D*D]
    in2 = [
        {"w2t": np.ascontiguousarray(W2T[:, c * JPC : (c + 1) * JPC]), "ht": hT}
        for c in cores
    ]
    r2 = run_bass_kernel_spmd(_get("params"), in2, core_ids=cores)
    params = np.empty((B, NROT * D * D), dtype=np.float32)
    for c in cores:
        params[:, c * JPC : (c + 1) * JPC] = r2.results[c]["paramsT"].T
    params += b2.astype(np.float32)

    # ---- host: skew-symmetrize ----
    P = params.reshape(B, NROT, D, D).astype(np.float64)
    G = 0.5 * (P - np.swapaxes(P, 2, 3))
    gnorm = max(
        np.linalg.norm(G[b, i], 2) for b in range(B) for i in range(NROT)
    )
    Gf = np.ascontiguousarray(G, dtype=np.float32)
    nGf = np.ascontiguousarray(-G, dtype=np.float32)

    if gnorm > 1.0:
        # Taylor-12 margin exceeded (never happens for the benchmark inputs);
        # fall back to exact host expm + device einsum-only path.
        return _fallback_host_expm(x, G)

    # ---- L3: expm + chain + einsum ----
    in3 = [
        {
            "x": x[c * BPC : (c + 1) * BPC],
            "g": Gf[c * BPC : (c + 1) * BPC],
            "ng": nGf[c * BPC : (c + 1) * BPC],
        }
        for c in cores
    ]
    r3 = run_bass_kernel_spmd(_get("rot"), in3, core_ids=cores)
    out = np.concatenate([r3.results[c]["y"] for c in cores], axis=0)
    return out


def _fallback_host_expm(x, G):
    from scipy.linalg import expm as _expm

    Rall = np.empty((B, D, D), dtype=np.float64)
    for b in range(B):
        R = np.eye(D)
        for i in range(NROT):
            R = R @ _expm(G[b, i])
        Rall[b] = R
    out = np.einsum("bnd,bde->bne", x.astype(np.float64), Rall)
    return out.astype(np.float32)


# revision 7
# speedup vs baseline: 1.0844x; 1.0844x over previous
"""Trainium2 Bass kernel for nn_DynamicGeometricRotation.

Reference computation (B=16, S=8192, D=128, H=512, R=3):
    pooled = x.mean(S)                           [B, D]
    h      = gelu_exact(pooled @ W1.T + b1)      [B, H]
    params = (h @ W2.T + b2) -> [B, R, D, D]
    for i in 0..R:  g_i = 0.5(p_i - p_i^T);  x = x @ expm(g_i)

Key identity: the rotations depend only on the ORIGINAL x (pooled before the
loop), so out = x @ (R1 @ R2 @ R3) — a single batched einsum.

Device plan (8 cores):
  L1 "pool":   batch-sharded (2 batches/core). Column-sums of x via PE
               (x-tile stationary, ones moving), accumulate in PSUM.
  host:        tiny MLP (pooled @ W1.T + b1, exact-erf gelu) in f64.
  L2 "params": W2 column-sharded (6144 rows of the 49152-row output per
               core, 12 MB instead of 96 MB each). paramsT = W2T_c.T-chunks
               as PE stationary, hT moving.
  host:        add b2, skew-symmetrize -> G (and -G).
  L3 "rot":    batch-sharded. On-device expm via degree-12 Taylor in
               Paterson-Stockmeyer form (5 matmuls; all stationary operands
               are symmetric powers of the skew G, or -G, so no PE
               transposes are needed), rotation chain R1@R2@R3 (2 PE
               transposes), then the einsum out = x @ Rall with per-tile PE
               transposes of x.
"""

import math

import numpy as np

import concourse.bass as bass
import concourse.mybir as mybir
import concourse.tile as tile
from concourse.bass_utils import run_bass_kernel_spmd
from concourse.masks import make_identity

F32 = mybir.dt.float32

B, S, D = 16, 8192, 128
H = 512
NROT = 3
NCORES = 8
BPC = B // NCORES           # batches per core = 2
JPC = NROT * D * D // NCORES  # W2 output rows per core = 6144

_FACT_INV = [1.0 / math.factorial(k) for k in range(13)]


def _split_sync_waits(nc, max_waits=1):
    """walrus in this container rejects >1 semaphore wait per instruction
    ("Too many sync wait commands"). Split extra waits into preceding
    same-engine NOPs (the engine stalls at the NOP, preserving
    happens-before)."""
    for fn in nc.m.functions:
        for bb in fn.blocks:
            insts = bb.instructions
            i = 0
            while i < len(insts):
                inst = insts[i]
                si = inst.sync_info
                if si is not None and len(si.on_wait) > max_waits:
                    waits = list(si.on_wait)
                    keep = waits[-max_waits:]
                    rest = waits[:-max_waits]
                    nops = []
                    for j in range(0, len(rest), max_waits):
                        nops.append(
                            mybir.InstNoOp(
                                name=f"{inst.name}-waitsplit-{j}",
                                engine=inst.engine,
                                sync_info=mybir.SyncInfo(
                                    on_wait=rest[j : j + max_waits], on_update=[]
                                ),
                                bass_nofuse=True,
                            )
                        )
                    inst.sync_info = mybir.SyncInfo(
                        on_wait=keep, on_update=list(si.on_update)
                    )
                    for k, nop in enumerate(nops):
                        insts.insert(i + k, nop)
                    i += len(nops)
                i += 1
    return nc


def build_pool():
    """Per core: x [BPC, S, D] -> pooledT [D, BPC] (sum over S)."""
    nc = bass.Bass(target_bir_lowering=False)
    x = nc.declare_dram_parameter("x", [BPC, S, D], F32, isOutput=False)
    out = nc.declare_dram_parameter("pooledT", [D, BPC], F32, isOutput=True)
    # [b, c, t, p, d]: chunk c = 1024 rows = 8 tiles of 128
    xr = x.rearrange("b (c t p) d -> b c p t d", p=128, t=8)
    nchunk = S // 1024
    with tile.TileContext(nc) as tc:
        with (
            tc.tile_pool(name="xin", bufs=3) as xpool,
            tc.tile_pool(name="one", bufs=1) as spool,
            tc.tile_pool(name="acc", bufs=1, space="PSUM") as pspool,
        ):
            ones = spool.tile([128, 1], F32)
            nc.vector.memset(ones, 1.0)
            acc = pspool.tile([128, BPC], F32)
            res = spool.tile([128, BPC], F32)
            for b in range(BPC):
                for c in range(nchunk):
                    xt = xpool.tile([128, 8, 128], F32)
                    nc.sync.dma_start(out=xt, in_=xr[b, c])
                    for t in range(8):
                        nc.tensor.matmul(
                            acc[:, b : b + 1],
                            lhsT=xt[:, t, :],
                            rhs=ones,
                            start=(c == 0 and t == 0),
                            stop=(c == nchunk - 1 and t == 7),
                        )
            nc.vector.tensor_copy(res, acc)
            nc.sync.dma_start(out=out[:, :], in_=res)
    return _split_sync_waits(nc)


def build_params():
    """Per core: paramsT[j, b] = sum_k W2T_c[k, j] * hT[k, b].
    W2T_c = W2.T[:, c*JPC:(c+1)*JPC]  ([H, JPC], 12 MB)."""
    nc = bass.Bass(target_bir_lowering=False)
    w2t = nc.declare_dram_parameter("w2t", [H, JPC], F32, isOutput=False)
    ht = nc.declare_dram_parameter("ht", [H, B], F32, isOutput=False)
    out = nc.declare_dram_parameter("paramsT", [JPC, B], F32, isOutput=True)
    KT = H // 128          # 4 k-tiles
    JI = 12                # j-tiles per group
    JO = JPC // (JI * 128)  # 4 groups of 1536 columns
    htr = ht.rearrange("(t p) b -> p t b", p=128)
    outr = out.rearrange("(jo ji p) b -> jo p ji b", p=128, ji=JI)
    with tile.TileContext(nc) as tc:
        with (
            tc.tile_pool(name="w", bufs=1) as wpool,
            tc.tile_pool(name="h", bufs=1) as hpool,
            tc.tile_pool(name="o", bufs=2) as opool,
            tc.tile_pool(name="ps", bufs=2, space="PSUM") as pspool,
        ):
            ht_sb = hpool.tile([128, KT, B], F32)
            nc.sync.dma_start(out=ht_sb, in_=htr)
            # preload the whole 12 MB W2T slice (4 chunks of [128, 6144]) so
            # the 192 matmuls run back-to-back behind the DMA stream
            ws = []
            for k in range(KT):
                w = wpool.tile([128, JPC], F32, tag=f"w{k}")
                nc.sync.dma_start(out=w, in_=w2t[k * 128 : (k + 1) * 128, :])
                ws.append(w)
            for jo in range(JO):
                ps = pspool.tile([128, JI, B], F32)
                # accumulate each psum slice k-contiguously (interleaving
                # accumulation groups within one PSUM bank corrupts the
                # bank-granular has_written state).
                for ji in range(JI):
                    j0 = (jo * JI + ji) * 128
                    for k in range(KT):
                        nc.tensor.matmul(
                            ps[:, ji, :],
                            lhsT=ws[k][:, j0 : j0 + 128],
                            rhs=ht_sb[:, k, :],
                            start=(k == 0),
                            stop=(k == KT - 1),
                        )
                res = opool.tile([128, JI, B], F32)
                nc.vector.tensor_copy(res, ps)
                nc.sync.dma_start(out=outr[jo], in_=res)
    return _split_sync_waits(nc)


def build_rot():
    """Per core: x [BPC, S, D], g/ng [BPC, NROT, D, D] -> y = x @ expm-chain.

    expm(G) = degree-12 Taylor, Paterson-Stockmeyer in powers of G4:
      T = P0 + G4 @ (P1 + G4 @ (P2 + c12*G4)),  Pk = polys of I, G, G2, G3.
    All PE stationary operands are -G (skew) or the symmetric G2/G4, so the
    expm needs no transposes. Elementwise Taylor combos are batched across
    the 3 rotations as [128, 3*128] DVE ops. The einsum batches 4 transposed
    tiles / 4 matmul outputs per PSUM bank so each bank drains with a single
    [128, 512] copy (DVE for xT, ACT for y).
    """
    nc = bass.Bass(target_bir_lowering=False)
    x = nc.declare_dram_parameter("x", [BPC, S, D], F32, isOutput=False)
    g = nc.declare_dram_parameter("g", [BPC, NROT, D, D], F32, isOutput=False)
    ng = nc.declare_dram_parameter("ng", [BPC, NROT, D, D], F32, isOutput=False)
    y = nc.declare_dram_parameter("y", [BPC, S, D], F32, isOutput=True)

    CH = 512               # einsum chunk rows
    TPC = CH // 128        # tiles per chunk
    nchunk = S // CH
    xr = x.rearrange("b (c t p) d -> b c p t d", p=128, t=TPC)
    yr = y.rearrange("b (c t p) d -> b c p t d", p=128, t=TPC)
    gr = g.rearrange("b r p d -> b p r d")
    ngr = ng.rearrange("b r p d -> b p r d")

    C = _FACT_INV
    A = mybir.AluOpType

    with tile.TileContext(nc) as tc:
        with (
            tc.tile_pool(name="const", bufs=1) as cpool,
            tc.tile_pool(name="gin", bufs=2) as gpool,
            tc.tile_pool(name="expm", bufs=2) as epool,
            tc.tile_pool(name="rmat", bufs=2) as rpool,
            tc.tile_pool(name="xin", bufs=3) as xpool,
            tc.tile_pool(name="xts", bufs=3) as xtpool,
            tc.tile_pool(name="yout", bufs=3) as ypool,
            tc.tile_pool(name="psT", bufs=2, space="PSUM") as psT,
            tc.tile_pool(name="psY", bufs=2, space="PSUM") as psY,
            tc.tile_pool(name="psE", bufs=2, space="PSUM") as psE,
        ):
            ident = cpool.tile([128, 128], F32)
            make_identity(nc, ident)
            ident3 = cpool.tile([128, NROT, 128], F32)
            for i in range(NROT):
                nc.vector.tensor_copy(ident3[:, i, :], ident)

            rall_tiles = []
            for b in range(BPC):
                g_sb = gpool.tile([128, NROT, 128], F32, tag="g_sb")
                ng_sb = gpool.tile([128, NROT, 128], F32, tag="ng_sb")
                nc.sync.dma_start(out=g_sb, in_=gr[b])
                nc.sync.dma_start(out=ng_sb, in_=ngr[b])

                def pow_mm(dst_tag, lhs_of, rhs_of):
                    ps = psE.tile([128, NROT, 128], F32, tag="ep")
                    for i in range(NROT):
                        nc.tensor.matmul(
                            ps[:, i, :], lhsT=lhs_of(i), rhs=rhs_of(i),
                            start=True, stop=True,
                        )
                    dst = epool.tile([128, NROT, 128], F32, tag=dst_tag)
                    nc.vector.tensor_copy(dst, ps)
                    return dst

                g2 = pow_mm("g2", lambda i: ng_sb[:, i, :], lambda i: g_sb[:, i, :])
                g3 = pow_mm("g3", lambda i: g2[:, i, :], lambda i: g_sb[:, i, :])
                g4 = pow_mm("g4", lambda i: g2[:, i, :], lambda i: g2[:, i, :])

                # inner = c12*G4 + c8 I + c9 G + c10 G2 + c11 G3
                t1 = epool.tile([128, NROT, 128], F32, tag="t1")
                nc.vector.tensor_scalar_mul(t1, g3, C[11])
                nc.vector.scalar_tensor_tensor(t1, g4, C[12], t1, A.mult, A.add)
                nc.vector.scalar_tensor_tensor(t1, g2, C[10], t1, A.mult, A.add)
                nc.vector.scalar_tensor_tensor(t1, g_sb, C[9], t1, A.mult, A.add)
                nc.vector.scalar_tensor_tensor(t1, ident3, C[8], t1, A.mult, A.add)
                u1p = psE.tile([128, NROT, 128], F32, tag="ep")
                for i in range(NROT):
                    nc.tensor.matmul(u1p[:, i, :], lhsT=g4[:, i, :], rhs=t1[:, i, :],
                                     start=True, stop=True)
                # V = U1 + c4 I + c5 G + c6 G2 + c7 G3
                t2 = epool.tile([128, NROT, 128], F32, tag="t2")
                nc.vector.tensor_scalar_mul(t2, g3, C[7])
                nc.vector.scalar_tensor_tensor(t2, g2, C[6], t2, A.mult, A.add)
                nc.vector.scalar_tensor_tensor(t2, g_sb, C[5], t2, A.mult, A.add)
                nc.vector.scalar_tensor_tensor(t2, ident3, C[4], t2, A.mult, A.add)
                nc.vector.tensor_tensor(t2, t2, u1p, A.add)
                u2p = psE.tile([128, NROT, 128], F32, tag="ep")
                for i in range(NROT):
                    nc.tensor.matmul(u2p[:, i, :], lhsT=g4[:, i, :], rhs=t2[:, i, :],
                                     start=True, stop=True)
                # R = U2 + I + G + G2/2 + G3/6
                t3 = epool.tile([128, NROT, 128], F32, tag="t3")
                nc.vector.tensor_scalar_mul(t3, g3, C[3])
                nc.vector.scalar_tensor_tensor(t3, g2, C[2], t3, A.mult, A.add)
                nc.vector.tensor_tensor(t3, t3, g_sb, A.add)
                nc.vector.tensor_tensor(t3, t3, ident3, A.add)
                r_sb = rpool.tile([128, NROT, 128], F32, tag="r_sb")
                nc.vector.tensor_tensor(r_sb, t3, u2p, A.add)

                # chain: Rall = R0 @ R1 @ R2
                t1p = psE.tile([128, 128], F32, tag="ep")
                nc.tensor.transpose(t1p, r_sb[:, 0, :], ident)
                r0t = epool.tile([128, 128], F32, tag="r0t")
                nc.vector.tensor_copy(r0t, t1p)
                r01p = psE.tile([128, 128], F32, tag="ep")
                nc.tensor.matmul(r01p, lhsT=r0t, rhs=r_sb[:, 1, :], start=True, stop=True)
                r01 = epool.tile([128, 128], F32, tag="r01")
                nc.vector.tensor_copy(r01, r01p)
                t2p = psE.tile([128, 128], F32, tag="ep")
                nc.tensor.transpose(t2p, r01, ident)
                r01t = epool.tile([128, 128], F32, tag="r01t")
                nc.vector.tensor_copy(r01t, t2p)
                rallp = psE.tile([128, 128], F32, tag="ep")
                nc.tensor.matmul(rallp, lhsT=r01t, rhs=r_sb[:, 2, :], start=True, stop=True)
                rall = rpool.tile([128, 128], F32, tag="rall")
                nc.vector.tensor_copy(rall, rallp)
                rall_tiles.append(rall)

            for b in range(BPC):
                rall = rall_tiles[b]
                for c in range(nchunk):
                    xt = xpool.tile([128, TPC, 128], F32, tag="xt")
                    nc.sync.dma_start(out=xt, in_=xr[b, c])
                    tp = psT.tile([128, TPC, 128], F32, tag="tp")
                    for t in range(TPC):
                        nc.tensor.transpose(tp[:, t, :], xt[:, t, :], ident)
                    xts = xtpool.tile([128, TPC, 128], F32, tag="xts")
                    nc.vector.tensor_copy(xts, tp)
                    yp = psY.tile([128, TPC, 128], F32, tag="yp")
                    for t in range(TPC):
                        nc.tensor.matmul(yp[:, t, :], lhsT=xts[:, t, :], rhs=rall,
                                         start=True, stop=True)
                    yt = ypool.tile([128, TPC, 128], F32, tag="yt")
                    nc.scalar.copy(yt, yp)
                    nc.sync.dma_start(out=yr[b, c], in_=yt)
    return _split_sync_waits(nc)


_CACHE = {}


def _get(name):
    if name not in _CACHE:
        _CACHE[name] = {"pool": build_pool, "params": build_params, "rot": build_rot}[
            name
        ]()
    return _CACHE[name]


def _erf(z):
    from scipy.special import erf

    return erf(z)


def kernel(x, W1, b1, W2, b2):
    x = np.ascontiguousarray(x, dtype=np.float32)
    cores = list(range(NCORES))

    # ---- L1: pooling ----
    in1 = [{"x": x[c * BPC : (c + 1) * BPC]} for c in cores]
    r1 = run_bass_kernel_spmd(_get("pool"), in1, core_ids=cores)
    pooled = np.concatenate(
        [r1.results[c]["pooledT"].T for c in cores], axis=0
    ).astype(np.float64) / float(S)                     # [B, D]

    # ---- host: tiny MLP with exact-erf gelu ----
    pre = pooled @ W1.astype(np.float64).T + b1.astype(np.float64)
    hh = 0.5 * pre * (1.0 + _erf(pre / np.sqrt(2.0)))
    hT = np.ascontiguousarray(hh.T, dtype=np.float32)   # [H, B]

    # ---- L2: params = h @ W2.T (sharded over W2 rows) ----
    W2T = np.ascontiguousarray(W2.astype(np.float32).T)  # [H, NROT*D*D]
    in2 = [
        {"w2t": np.ascontiguousarray(W2T[:, c * JPC : (c + 1) * JPC]), "ht": hT}
        for c in cores
    ]
    r2 = run_bass_kernel_spmd(_get("params"), in2, core_ids=cores)
    params = np.empty((B, NROT * D * D), dtype=np.float32)
    for c in cores:
        params[:, c * JPC : (c + 1) * JPC] = r2.results[c]["paramsT"].T
    params += b2.astype(np.float32)

    # ---- host: skew-symmetrize ----
    P = params.reshape(B, NROT, D, D).astype(np.float64)
    G = 0.5 * (P - np.swapaxes(P, 2, 3))
    gnorm = max(
        np.linalg.norm(G[b, i], 2) for b in range(B) for i in range(NROT)
    )
    Gf = np.ascontiguousarray(G, dtype=np.float32)
    nGf = np.ascontiguousarray(-G, dtype=np.float32)

    if gnorm > 1.0:
        # Taylor-12 margin exceeded (never happens for the benchmark inputs);
        # fall back to exact host expm + device einsum-only path.
        return _fallback_host_expm(x, G)

    # ---- L3: expm + chain + einsum ----
    in3 = [
        {
            "x": x[c * BPC : (c + 1) * BPC],
            "g": Gf[c * BPC : (c + 1) * BPC],
            "ng": nGf[c * BPC : (c + 1) * BPC],
        }
        for c in cores
    ]
    r3 = run_bass_kernel_spmd(_get("rot"), in3, core_ids=cores)
    out = np.concatenate([r3.results[c]["y"] for c in cores], axis=0)
    return out


def _fallback_host_expm(x, G):
    from scipy.linalg import expm as _expm

    Rall = np.empty((B, D, D), dtype=np.float64)
    for b in range(B):
        R = np.eye(D)
        for i in range(NROT):
            R = R @ _expm(G[b, i])
        Rall[b] = R
    out = np.einsum("bnd,bde->bne", x.astype(np.float64), Rall)
    return out.astype(np.float32)


# revision 9
# speedup vs baseline: 1.1151x; 1.0283x over previous
"""Trainium2 Bass kernel for nn_DynamicGeometricRotation.

Reference computation (B=16, S=8192, D=128, H=512, R=3):
    pooled = x.mean(S)                           [B, D]
    h      = gelu_exact(pooled @ W1.T + b1)      [B, H]
    params = (h @ W2.T + b2) -> [B, R, D, D]
    for i in 0..R:  g_i = 0.5(p_i - p_i^T);  x = x @ expm(g_i)

Key identity: the rotations depend only on the ORIGINAL x (pooled before the
loop), so out = x @ (R1 @ R2 @ R3) — a single batched einsum.

Device plan (8 cores):
  L1 "pool":   batch-sharded (2 batches/core). Column-sums of x via PE
               (x-tile stationary, ones moving), accumulate in PSUM.
  host:        tiny MLP (pooled @ W1.T + b1, exact-erf gelu) in f64.
  L2 "params": W2 column-sharded (6144 rows of the 49152-row output per
               core, 12 MB instead of 96 MB each). paramsT = W2T_c.T-chunks
               as PE stationary, hT moving.
  host:        add b2, skew-symmetrize -> G (and -G).
  L3 "rot":    batch-sharded. On-device expm via degree-12 Taylor in
               Paterson-Stockmeyer form (5 matmuls; all stationary operands
               are symmetric powers of the skew G, or -G, so no PE
               transposes are needed), rotation chain R1@R2@R3 (2 PE
               transposes), then the einsum out = x @ Rall with per-tile PE
               transposes of x.
"""

import math

import numpy as np

import concourse.bass as bass
import concourse.mybir as mybir
import concourse.tile as tile
from concourse.bass_utils import run_bass_kernel_spmd
from concourse.masks import make_identity

F32 = mybir.dt.float32

B, S, D = 16, 8192, 128
H = 512
NROT = 3
NCORES = 8
BPC = B // NCORES           # batches per core = 2
JPC = NROT * D * D // NCORES  # W2 output rows per core = 6144

_FACT_INV = [1.0 / math.factorial(k) for k in range(13)]


def _split_sync_waits(nc, max_waits=1):
    """walrus in this container rejects >1 semaphore wait per instruction
    ("Too many sync wait commands"). Split extra waits into preceding
    same-engine NOPs (the engine stalls at the NOP, preserving
    happens-before)."""
    for fn in nc.m.functions:
        for bb in fn.blocks:
            insts = bb.instructions
            i = 0
            while i < len(insts):
                inst = insts[i]
                si = inst.sync_info
                if si is not None and len(si.on_wait) > max_waits:
                    waits = list(si.on_wait)
                    keep = waits[-max_waits:]
                    rest = waits[:-max_waits]
                    nops = []
                    for j in range(0, len(rest), max_waits):
                        nops.append(
                            mybir.InstNoOp(
                                name=f"{inst.name}-waitsplit-{j}",
                                engine=inst.engine,
                                sync_info=mybir.SyncInfo(
                                    on_wait=rest[j : j + max_waits], on_update=[]
                                ),
                                bass_nofuse=True,
                            )
                        )
                    inst.sync_info = mybir.SyncInfo(
                        on_wait=keep, on_update=list(si.on_update)
                    )
                    for k, nop in enumerate(nops):
                        insts.insert(i + k, nop)
                    i += len(nops)
                i += 1
    return nc


def build_pool():
    """Per core: x [BPC, S, D] -> pooledT [D, BPC] (sum over S)."""
    nc = bass.Bass(target_bir_lowering=False)
    x = nc.declare_dram_parameter("x", [BPC, S, D], F32, isOutput=False)
    out = nc.declare_dram_parameter("pooledT", [D, BPC], F32, isOutput=True)
    # [b, c, t, p, d]: chunk c = 1024 rows = 8 tiles of 128
    xr = x.rearrange("b (c t p) d -> b c p t d", p=128, t=8)
    nchunk = S // 1024
    with tile.TileContext(nc) as tc:
        with (
            tc.tile_pool(name="xin", bufs=3) as xpool,
            tc.tile_pool(name="one", bufs=1) as spool,
            tc.tile_pool(name="acc", bufs=1, space="PSUM") as pspool,
        ):
            ones = spool.tile([128, 1], F32)
            nc.vector.memset(ones, 1.0)
            acc = pspool.tile([128, BPC], F32)
            res = spool.tile([128, BPC], F32)
            for b in range(BPC):
                for c in range(nchunk):
                    xt = xpool.tile([128, 8, 128], F32)
                    nc.sync.dma_start(out=xt, in_=xr[b, c])
                    for t in range(8):
                        nc.tensor.matmul(
                            acc[:, b : b + 1],
                            lhsT=xt[:, t, :],
                            rhs=ones,
                            start=(c == 0 and t == 0),
                            stop=(c == nchunk - 1 and t == 7),
                        )
            nc.vector.tensor_copy(res, acc)
            nc.sync.dma_start(out=out[:, :], in_=res)
    return _split_sync_waits(nc)


def build_params():
    """Per core: paramsT[j, b] = sum_k W2T_c[k, j] * hT[k, b].
    W2T_c = W2.T[:, c*JPC:(c+1)*JPC]  ([H, JPC], 12 MB)."""
    nc = bass.Bass(target_bir_lowering=False)
    w2t = nc.declare_dram_parameter("w2t", [H, JPC], F32, isOutput=False)
    ht = nc.declare_dram_parameter("ht", [H, B], F32, isOutput=False)
    out = nc.declare_dram_parameter("paramsT", [JPC, B], F32, isOutput=True)
    KT = H // 128          # 4 k-tiles
    JI = 12                # j-tiles per group
    JO = JPC // (JI * 128)  # 4 groups of 1536 columns
    htr = ht.rearrange("(t p) b -> p t b", p=128)
    outr = out.rearrange("(jo ji p) b -> jo p ji b", p=128, ji=JI)
    w2tr = w2t.rearrange("(kt p) j -> p kt j", p=128)
    with tile.TileContext(nc) as tc:
        with (
            tc.tile_pool(name="w", bufs=3) as wpool,
            tc.tile_pool(name="h", bufs=1) as hpool,
            tc.tile_pool(name="o", bufs=2) as opool,
            tc.tile_pool(name="ps", bufs=2, space="PSUM") as pspool,
        ):
            ht_sb = hpool.tile([128, KT, B], F32)
            nc.sync.dma_start(out=ht_sb, in_=htr)
            # stream W2T in k-complete column panels [all 512 k, 1536 j] so
            # each psum group can finish as soon as its own panel lands
            for jo in range(JO):
                w = wpool.tile([128, KT, JI * 128], F32, tag="w")
                nc.sync.dma_start(
                    out=w, in_=w2tr[:, :, jo * JI * 128 : (jo + 1) * JI * 128]
                )
                ps = pspool.tile([128, JI, B], F32)
                # accumulate each psum slice k-contiguously (interleaving
                # accumulation groups within one PSUM bank corrupts the
                # bank-granular has_written state).
                for ji in range(JI):
                    for k in range(KT):
                        nc.tensor.matmul(
                            ps[:, ji, :],
                            lhsT=w[:, k, ji * 128 : (ji + 1) * 128],
                            rhs=ht_sb[:, k, :],
                            start=(k == 0),
                            stop=(k == KT - 1),
                        )
                res = opool.tile([128, JI, B], F32)
                nc.vector.tensor_copy(res, ps)
                nc.sync.dma_start(out=outr[jo], in_=res)
    return _split_sync_waits(nc)


def build_rot():
    """Per core: x [BPC, S, D], g/ng [BPC, NROT, D, D] -> y = x @ expm-chain.

    expm(G) = degree-12 Taylor, Paterson-Stockmeyer in powers of G4:
      T = P0 + G4 @ (P1 + G4 @ (P2 + c12*G4)),  Pk = polys of I, G, G2, G3.
    All PE stationary operands are -G (skew) or the symmetric G2/G4, so the
    expm needs no transposes. Elementwise Taylor combos are batched across
    the 3 rotations as [128, 3*128] DVE ops. The einsum batches 4 transposed
    tiles / 4 matmul outputs per PSUM bank so each bank drains with a single
    [128, 512] copy (DVE for xT, ACT for y).
    """
    nc = bass.Bass(target_bir_lowering=False)
    x = nc.declare_dram_parameter("x", [BPC, S, D], F32, isOutput=False)
    g = nc.declare_dram_parameter("g", [BPC, NROT, D, D], F32, isOutput=False)
    ng = nc.declare_dram_parameter("ng", [BPC, NROT, D, D], F32, isOutput=False)
    y = nc.declare_dram_parameter("y", [BPC, S, D], F32, isOutput=True)

    CH = 512               # einsum chunk rows
    TPC = CH // 128        # tiles per chunk
    nchunk = S // CH
    xr = x.rearrange("b (c t p) d -> b c p t d", p=128, t=TPC)
    yr = y.rearrange("b (c t p) d -> b c p t d", p=128, t=TPC)
    gr = g.rearrange("b r p d -> b p r d")
    ngr = ng.rearrange("b r p d -> b p r d")

    C = _FACT_INV
    A = mybir.AluOpType

    with tile.TileContext(nc) as tc:
        with (
            tc.tile_pool(name="const", bufs=1) as cpool,
            tc.tile_pool(name="gin", bufs=2) as gpool,
            tc.tile_pool(name="expm", bufs=2) as epool,
            tc.tile_pool(name="rmat", bufs=2) as rpool,
            tc.tile_pool(name="xin", bufs=3) as xpool,
            tc.tile_pool(name="xts", bufs=3) as xtpool,
            tc.tile_pool(name="yout", bufs=3) as ypool,
            tc.tile_pool(name="psT", bufs=2, space="PSUM") as psT,
            tc.tile_pool(name="psY", bufs=2, space="PSUM") as psY,
            tc.tile_pool(name="psE", bufs=2, space="PSUM") as psE,
        ):
            ident = cpool.tile([128, 128], F32)
            make_identity(nc, ident)
            ident3 = cpool.tile([128, NROT, 128], F32)
            for i in range(NROT):
                nc.vector.tensor_copy(ident3[:, i, :], ident)

            rall_tiles = []
            for b in range(BPC):
                g_sb = gpool.tile([128, NROT, 128], F32, tag="g_sb")
                ng_sb = gpool.tile([128, NROT, 128], F32, tag="ng_sb")
                nc.sync.dma_start(out=g_sb, in_=gr[b])
                nc.sync.dma_start(out=ng_sb, in_=ngr[b])

                def pow_mm(dst_tag, lhs_of, rhs_of):
                    ps = psE.tile([128, NROT, 128], F32, tag="ep")
                    for i in range(NROT):
                        nc.tensor.matmul(
                            ps[:, i, :], lhsT=lhs_of(i), rhs=rhs_of(i),
                            start=True, stop=True,
                        )
                    dst = epool.tile([128, NROT, 128], F32, tag=dst_tag)
                    nc.vector.tensor_copy(dst, ps)
                    return dst

                g2 = pow_mm("g2", lambda i: ng_sb[:, i, :], lambda i: g_sb[:, i, :])
                g3 = pow_mm("g3", lambda i: g2[:, i, :], lambda i: g_sb[:, i, :])
                g4 = pow_mm("g4", lambda i: g2[:, i, :], lambda i: g2[:, i, :])

                # inner = c12*G4 + c8 I + c9 G + c10 G2 + c11 G3
                t1 = epool.tile([128, NROT, 128], F32, tag="t1")
                nc.vector.tensor_scalar_mul(t1, g3, C[11])
                nc.vector.scalar_tensor_tensor(t1, g4, C[12], t1, A.mult, A.add)
                nc.vector.scalar_tensor_tensor(t1, g2, C[10], t1, A.mult, A.add)
                nc.vector.scalar_tensor_tensor(t1, g_sb, C[9], t1, A.mult, A.add)
                nc.vector.scalar_tensor_tensor(t1, ident3, C[8], t1, A.mult, A.add)
                u1p = psE.tile([128, NROT, 128], F32, tag="ep")
                for i in range(NROT):
                    nc.tensor.matmul(u1p[:, i, :], lhsT=g4[:, i, :], rhs=t1[:, i, :],
                                     start=True, stop=True)
                # V = U1 + c4 I + c5 G + c6 G2 + c7 G3
                t2 = epool.tile([128, NROT, 128], F32, tag="t2")
                nc.vector.tensor_scalar_mul(t2, g3, C[7])
                nc.vector.scalar_tensor_tensor(t2, g2, C[6], t2, A.mult, A.add)
                nc.vector.scalar_tensor_tensor(t2, g_sb, C[5], t2, A.mult, A.add)
                nc.vector.scalar_tensor_tensor(t2, ident3, C[4], t2, A.mult, A.add)
                nc.vector.tensor_tensor(t2, t2, u1p, A.add)
                u2p = psE.tile([128, NROT, 128], F32, tag="ep")
                for i in range(NROT):
                    nc.tensor.matmul(u2p[:, i, :], lhsT=g4[:, i, :], rhs=t2[:, i, :],
                                     start=True, stop=True)
                # R = U2 + I + G + G2/2 + G3/6
                t3 = epool.tile([128, NROT, 128], F32, tag="t3")
                nc.vector.tensor_scalar_mul(t3, g3, C[3])
                nc.vector.scalar_tensor_tensor(t3, g2, C[2], t3, A.mult, A.add)
                nc.vector.tensor_tensor(t3, t3, g_sb, A.add)
                nc.vector.tensor_tensor(t3, t3, ident3, A.add)
                r_sb = rpool.tile([128, NROT, 128], F32, tag="r_sb")
                nc.vector.tensor_tensor(r_sb, t3, u2p, A.add)

                # chain: Rall = R0 @ R1 @ R2
                t1p = psE.tile([128, 128], F32, tag="ep")
                nc.tensor.transpose(t1p, r_sb[:, 0, :], ident)
                r0t = epool.tile([128, 128], F32, tag="r0t")
                nc.vector.tensor_copy(r0t, t1p)
                r01p = psE.tile([128, 128], F32, tag="ep")
                nc.tensor.matmul(r01p, lhsT=r0t, rhs=r_sb[:, 1, :], start=True, stop=True)
                r01 = epool.tile([128, 128], F32, tag="r01")
                nc.vector.tensor_copy(r01, r01p)
                t2p = psE.tile([128, 128], F32, tag="ep")
                nc.tensor.transpose(t2p, r01, ident)
                r01t = epool.tile([128, 128], F32, tag="r01t")
                nc.vector.tensor_copy(r01t, t2p)
                rallp = psE.tile([128, 128], F32, tag="ep")
                nc.tensor.matmul(rallp, lhsT=r01t, rhs=r_sb[:, 2, :], start=True, stop=True)
                rall = rpool.tile([128, 128], F32, tag="rall")
                nc.vector.tensor_copy(rall, rallp)
                rall_tiles.append(rall)

            # Software-pipelined einsum: emit chunk i's transposes before
            # chunk i-1's matmuls so the PE never idles waiting for the DVE
            # psum->sbuf copy of the chunk it just transposed.
            chunks = [(b, c) for b in range(BPC) for c in range(nchunk)]
            staged = []  # (b, c, xts)
            for i in range(len(chunks) + 1):
                if i < len(chunks):
                    b, c = chunks[i]
                    xt = xpool.tile([128, TPC, 128], F32, tag="xt")
                    nc.sync.dma_start(out=xt, in_=xr[b, c])
                    tp = psT.tile([128, TPC, 128], F32, tag="tp")
                    for t in range(TPC):
                        nc.tensor.transpose(tp[:, t, :], xt[:, t, :], ident)
                    xts = xtpool.tile([128, TPC, 128], F32, tag="xts")
                    nc.vector.tensor_copy(xts, tp)
                    staged.append((b, c, xts))
                if i >= 1:
                    b, c, xts = staged[i - 1]
                    yp = psY.tile([128, TPC, 128], F32, tag="yp")
                    for t in range(TPC):
                        nc.tensor.matmul(yp[:, t, :], lhsT=xts[:, t, :],
                                         rhs=rall_tiles[b], start=True, stop=True)
                    yt = ypool.tile([128, TPC, 128], F32, tag="yt")
                    nc.scalar.copy(yt, yp)
                    nc.sync.dma_start(out=yr[b, c], in_=yt)
    return _split_sync_waits(nc)


_CACHE = {}


def _get(name):
    if name not in _CACHE:
        _CACHE[name] = {"pool": build_pool, "params": build_params, "rot": build_rot}[
            name
        ]()
    return _CACHE[name]


def _erf(z):
    from scipy.special import erf

    return erf(z)


def kernel(x, W1, b1, W2, b2):
    x = np.ascontiguousarray(x, dtype=np.float32)
    cores = list(range(NCORES))

    # ---- L1: pooling ----
    in1 = [{"x": x[c * BPC : (c + 1) * BPC]} for c in cores]
    r1 = run_bass_kernel_spmd(_get("pool"), in1, core_ids=cores)
    pooled = np.concatenate(
        [r1.results[c]["pooledT"].T for c in cores], axis=0
    ).astype(np.float64) / float(S)                     # [B, D]

    # ---- host: tiny MLP with exact-erf gelu ----
    pre = pooled @ W1.astype(np.float64).T + b1.astype(np.float64)
    hh = 0.5 * pre * (1.0 + _erf(pre / np.sqrt(2.0)))
    hT = np.ascontiguousarray(hh.T, dtype=np.float32)   # [H, B]

    # ---- L2: params = h @ W2.T (sharded over W2 rows) ----
    W2T = np.ascontiguousarray(W2.astype(np.float32).T)  # [H, NROT*D*D]
    in2 = [
        {"w2t": np.ascontiguousarray(W2T[:, c * JPC : (c + 1) * JPC]), "ht": hT}
        for c in cores
    ]
    r2 = run_bass_kernel_spmd(_get("params"), in2, core_ids=cores)
    params = np.empty((B, NROT * D * D), dtype=np.float32)
    for c in cores:
        params[:, c * JPC : (c + 1) * JPC] = r2.results[c]["paramsT"].T
    params += b2.astype(np.float32)

    # ---- host: skew-symmetrize ----
    P = params.reshape(B, NROT, D, D).astype(np.float64)
    G = 0.5 * (P - np.swapaxes(P, 2, 3))
    gnorm = max(
        np.linalg.norm(G[b, i], 2) for b in range(B) for i in range(NROT)
    )
    Gf = np.ascontiguousarray(G, dtype=np.float32)
    nGf = np.ascontiguousarray(-G, dtype=np.float32)

    if gnorm > 1.0:
        # Taylor-12 margin exceeded (never happens for the benchmark inputs);
        # fall back to exact host expm + device einsum-only path.
        return _fallback_host_expm(x, G)

    # ---- L3: expm + chain + einsum ----
    in3 = [
        {
            "x": x[c * BPC : (c + 1) * BPC],
            "g": Gf[c * BPC : (c + 1) * BPC],
            "ng": nGf[c * BPC : (c + 1) * BPC],
        }
        for c in cores
    ]
    r3 = run_bass_kernel_spmd(_get("rot"), in3, core_ids=cores)
    out = np.concatenate([r3.results[c]["y"] for c in cores], axis=0)
    return out


def _fallback_host_expm(x, G):
    from scipy.linalg import expm as _expm

    Rall = np.empty((B, D, D), dtype=np.float64)
    for b in range(B):
        R = np.eye(D)
        for i in range(NROT):
            R = R @ _expm(G[b, i])
        Rall[b] = R
    out = np.einsum("bnd,bde->bne", x.astype(np.float64), Rall)
    return out.astype(np.float32)


# revision 10
# speedup vs baseline: 1.1603x; 1.0405x over previous
"""Trainium2 Bass kernel for nn_DynamicGeometricRotation.

Reference computation (B=16, S=8192, D=128, H=512, R=3):
    pooled = x.mean(S)                           [B, D]
    h      = gelu_exact(pooled @ W1.T + b1)      [B, H]
    params = (h @ W2.T + b2) -> [B, R, D, D]
    for i in 0..R:  g_i = 0.5(p_i - p_i^T);  x = x @ expm(g_i)

Key identity: the rotations depend only on the ORIGINAL x (pooled before the
loop), so out = x @ (R1 @ R2 @ R3) — a single batched einsum.

Device plan (8 cores):
  L1 "pool":   batch-sharded (2 batches/core). Column-sums of x via PE
               (x-tile stationary, ones moving), accumulate in PSUM.
  host:        tiny MLP (pooled @ W1.T + b1, exact-erf gelu) in f64.
  L2 "params": W2 column-sharded (6144 rows of the 49152-row output per
               core, 12 MB instead of 96 MB each). paramsT = W2T_c.T-chunks
               as PE stationary, hT moving.
  host:        add b2, skew-symmetrize -> G (and -G).
  L3 "rot":    batch-sharded. On-device expm via degree-12 Taylor in
               Paterson-Stockmeyer form (5 matmuls; all stationary operands
               are symmetric powers of the skew G, or -G, so no PE
               transposes are needed), rotation chain R1@R2@R3 (2 PE
               transposes), then the einsum out = x @ Rall with per-tile PE
               transposes of x.
"""

import math

import numpy as np

import concourse.bass as bass
import concourse.mybir as mybir
import concourse.tile as tile
from concourse.bass_utils import run_bass_kernel_spmd
from concourse.masks import make_identity

F32 = mybir.dt.float32

B, S, D = 16, 8192, 128
H = 512
NROT = 3
NCORES = 8
BPC = B // NCORES           # batches per core = 2
JPC = NROT * D * D // NCORES  # W2 output rows per core = 6144

_FACT_INV = [1.0 / math.factorial(k) for k in range(13)]


def _split_sync_waits(nc, max_waits=1):
    """walrus in this container rejects >1 semaphore wait per instruction
    ("Too many sync wait commands"). Split extra waits into preceding
    same-engine NOPs (the engine stalls at the NOP, preserving
    happens-before)."""
    for fn in nc.m.functions:
        for bb in fn.blocks:
            insts = bb.instructions
            i = 0
            while i < len(insts):
                inst = insts[i]
                si = inst.sync_info
                if si is not None and len(si.on_wait) > max_waits:
                    waits = list(si.on_wait)
                    keep = waits[-max_waits:]
                    rest = waits[:-max_waits]
                    nops = []
                    for j in range(0, len(rest), max_waits):
                        nops.append(
                            mybir.InstNoOp(
                                name=f"{inst.name}-waitsplit-{j}",
                                engine=inst.engine,
                                sync_info=mybir.SyncInfo(
                                    on_wait=rest[j : j + max_waits], on_update=[]
                                ),
                                bass_nofuse=True,
                            )
                        )
                    inst.sync_info = mybir.SyncInfo(
                        on_wait=keep, on_update=list(si.on_update)
                    )
                    for k, nop in enumerate(nops):
                        insts.insert(i + k, nop)
                    i += len(nops)
                i += 1
    return nc


def build_pool():
    """Per core: x [BPC, S, D] -> pooledT [D, BPC] (sum over S)."""
    nc = bass.Bass(target_bir_lowering=False)
    x = nc.declare_dram_parameter("x", [BPC, S, D], F32, isOutput=False)
    out = nc.declare_dram_parameter("pooledT", [D, BPC], F32, isOutput=True)
    # [b, c, t, p, d]: chunk c = 1024 rows = 8 tiles of 128
    xr = x.rearrange("b (c t p) d -> b c p t d", p=128, t=8)
    nchunk = S // 1024
    with tile.TileContext(nc) as tc:
        with (
            tc.tile_pool(name="xin", bufs=5) as xpool,
            tc.tile_pool(name="one", bufs=1) as spool,
            tc.tile_pool(name="acc", bufs=1, space="PSUM") as pspool,
        ):
            ones = spool.tile([128, 1], F32)
            nc.vector.memset(ones, 1.0)
            acc = pspool.tile([128, BPC], F32)
            res = spool.tile([128, BPC], F32)
            for b in range(BPC):
                for c in range(nchunk):
                    xt = xpool.tile([128, 8, 128], F32)
                    nc.sync.dma_start(out=xt, in_=xr[b, c])
                    for t in range(8):
                        nc.tensor.matmul(
                            acc[:, b : b + 1],
                            lhsT=xt[:, t, :],
                            rhs=ones,
                            start=(c == 0 and t == 0),
                            stop=(c == nchunk - 1 and t == 7),
                        )
            nc.vector.tensor_copy(res, acc)
            nc.sync.dma_start(out=out[:, :], in_=res)
    return _split_sync_waits(nc)


def build_params():
    """Per core: paramsT[j, b] = sum_k W2T_c[k, j] * hT[k, b].
    W2T_c = W2.T[:, c*JPC:(c+1)*JPC]  ([H, JPC], 12 MB)."""
    nc = bass.Bass(target_bir_lowering=False)
    w2t = nc.declare_dram_parameter("w2t", [H, JPC], F32, isOutput=False)
    ht = nc.declare_dram_parameter("ht", [H, B], F32, isOutput=False)
    out = nc.declare_dram_parameter("paramsT", [JPC, B], F32, isOutput=True)
    KT = H // 128          # 4 k-tiles
    JI = 12                # j-tiles per group
    JO = JPC // (JI * 128)  # 4 groups of 1536 columns
    htr = ht.rearrange("(t p) b -> p t b", p=128)
    outr = out.rearrange("(jo ji p) b -> jo p ji b", p=128, ji=JI)
    w2tr = w2t.rearrange("(kt p) j -> p kt j", p=128)
    with tile.TileContext(nc) as tc:
        with (
            tc.tile_pool(name="w", bufs=3) as wpool,
            tc.tile_pool(name="h", bufs=1) as hpool,
            tc.tile_pool(name="o", bufs=2) as opool,
            tc.tile_pool(name="ps", bufs=2, space="PSUM") as pspool,
        ):
            ht_sb = hpool.tile([128, KT, B], F32)
            nc.sync.dma_start(out=ht_sb, in_=htr)
            # stream W2T in k-complete column panels [all 512 k, 1536 j] so
            # each psum group can finish as soon as its own panel lands
            for jo in range(JO):
                w = wpool.tile([128, KT, JI * 128], F32, tag="w")
                nc.sync.dma_start(
                    out=w, in_=w2tr[:, :, jo * JI * 128 : (jo + 1) * JI * 128]
                )
                ps = pspool.tile([128, JI, B], F32)
                # accumulate each psum slice k-contiguously (interleaving
                # accumulation groups within one PSUM bank corrupts the
                # bank-granular has_written state).
                for ji in range(JI):
                    for k in range(KT):
                        nc.tensor.matmul(
                            ps[:, ji, :],
                            lhsT=w[:, k, ji * 128 : (ji + 1) * 128],
                            rhs=ht_sb[:, k, :],
                            start=(k == 0),
                            stop=(k == KT - 1),
                        )
                res = opool.tile([128, JI, B], F32)
                nc.vector.tensor_copy(res, ps)
                nc.sync.dma_start(out=outr[jo], in_=res)
    return _split_sync_waits(nc)


def build_rot():
    """Per core: x [BPC, S, D], g/ng [BPC, NROT, D, D] -> y = x @ expm-chain.

    expm(G) = degree-12 Taylor, Paterson-Stockmeyer in powers of G4:
      T = P0 + G4 @ (P1 + G4 @ (P2 + c12*G4)),  Pk = polys of I, G, G2, G3.
    All PE stationary operands are -G (skew) or the symmetric G2/G4, so the
    expm needs no transposes. Elementwise Taylor combos are batched across
    the 3 rotations as [128, 3*128] DVE ops. The einsum batches 4 transposed
    tiles / 4 matmul outputs per PSUM bank so each bank drains with a single
    [128, 512] copy (DVE for xT, ACT for y).
    """
    nc = bass.Bass(target_bir_lowering=False)
    x = nc.declare_dram_parameter("x", [BPC, S, D], F32, isOutput=False)
    g = nc.declare_dram_parameter("g", [BPC, NROT, D, D], F32, isOutput=False)
    ng = nc.declare_dram_parameter("ng", [BPC, NROT, D, D], F32, isOutput=False)
    y = nc.declare_dram_parameter("y", [BPC, S, D], F32, isOutput=True)

    CH = 512               # einsum chunk rows
    TPC = CH // 128        # tiles per chunk
    nchunk = S // CH
    xr = x.rearrange("b (c t p) d -> b c p t d", p=128, t=TPC)
    yr = y.rearrange("b (c t p) d -> b c p t d", p=128, t=TPC)
    gr = g.rearrange("b r p d -> b p r d")
    ngr = ng.rearrange("b r p d -> b p r d")

    C = _FACT_INV
    A = mybir.AluOpType

    with tile.TileContext(nc) as tc:
        with (
            tc.tile_pool(name="const", bufs=1) as cpool,
            tc.tile_pool(name="gin", bufs=2) as gpool,
            tc.tile_pool(name="expm", bufs=2) as epool,
            tc.tile_pool(name="rmat", bufs=2) as rpool,
            tc.tile_pool(name="xin", bufs=5) as xpool,
            tc.tile_pool(name="xts", bufs=5) as xtpool,
            tc.tile_pool(name="yout", bufs=4) as ypool,
            tc.tile_pool(name="psT", bufs=3, space="PSUM") as psT,
            tc.tile_pool(name="psY", bufs=3, space="PSUM") as psY,
            tc.tile_pool(name="psE", bufs=2, space="PSUM") as psE,
        ):
            ident = cpool.tile([128, 128], F32)
            make_identity(nc, ident)
            ident3 = cpool.tile([128, NROT, 128], F32)
            for i in range(NROT):
                nc.vector.tensor_copy(ident3[:, i, :], ident)

            WARM = 3
            chunks = [(b, c) for b in range(BPC) for c in range(nchunk)]
            staged = []  # (b, c, xts)

            def stage_chunk(i):
                b, c = chunks[i]
                xt = xpool.tile([128, TPC, 128], F32, tag="xt")
                nc.sync.dma_start(out=xt, in_=xr[b, c])
                tp = psT.tile([128, TPC, 128], F32, tag="tp")
                for t in range(TPC):
                    nc.tensor.transpose(tp[:, t, :], xt[:, t, :], ident)
                xts = xtpool.tile([128, TPC, 128], F32, tag="xts")
                nc.vector.tensor_copy(xts, tp)
                staged.append((b, c, xts))

            for i in range(WARM):
                stage_chunk(i)

            rall_tiles = []
            for b in range(BPC):
                g_sb = gpool.tile([128, NROT, 128], F32, tag="g_sb")
                ng_sb = gpool.tile([128, NROT, 128], F32, tag="ng_sb")
                nc.sync.dma_start(out=g_sb, in_=gr[b])
                nc.sync.dma_start(out=ng_sb, in_=ngr[b])

                def pow_mm(dst_tag, lhs_of, rhs_of):
                    ps = psE.tile([128, NROT, 128], F32, tag="ep")
                    for i in range(NROT):
                        nc.tensor.matmul(
                            ps[:, i, :], lhsT=lhs_of(i), rhs=rhs_of(i),
                            start=True, stop=True,
                        )
                    dst = epool.tile([128, NROT, 128], F32, tag=dst_tag)
                    nc.vector.tensor_copy(dst, ps)
                    return dst

                g2 = pow_mm("g2", lambda i: ng_sb[:, i, :], lambda i: g_sb[:, i, :])
                g3 = pow_mm("g3", lambda i: g2[:, i, :], lambda i: g_sb[:, i, :])
                g4 = pow_mm("g4", lambda i: g2[:, i, :], lambda i: g2[:, i, :])

                # inner = c12*G4 + c8 I + c9 G + c10 G2 + c11 G3
                t1 = epool.tile([128, NROT, 128], F32, tag="t1")
                nc.vector.tensor_scalar_mul(t1, g3, C[11])
                nc.vector.scalar_tensor_tensor(t1, g4, C[12], t1, A.mult, A.add)
                nc.vector.scalar_tensor_tensor(t1, g2, C[10], t1, A.mult, A.add)
                nc.vector.scalar_tensor_tensor(t1, g_sb, C[9], t1, A.mult, A.add)
                nc.vector.scalar_tensor_tensor(t1, ident3, C[8], t1, A.mult, A.add)
                u1p = psE.tile([128, NROT, 128], F32, tag="ep")
                for i in range(NROT):
                    nc.tensor.matmul(u1p[:, i, :], lhsT=g4[:, i, :], rhs=t1[:, i, :],
                                     start=True, stop=True)
                # V = U1 + c4 I + c5 G + c6 G2 + c7 G3
                t2 = epool.tile([128, NROT, 128], F32, tag="t2")
                nc.vector.tensor_scalar_mul(t2, g3, C[7])
                nc.vector.scalar_tensor_tensor(t2, g2, C[6], t2, A.mult, A.add)
                nc.vector.scalar_tensor_tensor(t2, g_sb, C[5], t2, A.mult, A.add)
                nc.vector.scalar_tensor_tensor(t2, ident3, C[4], t2, A.mult, A.add)
                nc.vector.tensor_tensor(t2, t2, u1p, A.add)
                u2p = psE.tile([128, NROT, 128], F32, tag="ep")
                for i in range(NROT):
                    nc.tensor.matmul(u2p[:, i, :], lhsT=g4[:, i, :], rhs=t2[:, i, :],
                                     start=True, stop=True)
                # R = U2 + I + G + G2/2 + G3/6
                t3 = epool.tile([128, NROT, 128], F32, tag="t3")
                nc.vector.tensor_scalar_mul(t3, g3, C[3])
                nc.vector.scalar_tensor_tensor(t3, g2, C[2], t3, A.mult, A.add)
                nc.vector.tensor_tensor(t3, t3, g_sb, A.add)
                nc.vector.tensor_tensor(t3, t3, ident3, A.add)
                r_sb = rpool.tile([128, NROT, 128], F32, tag="r_sb")
                nc.vector.tensor_tensor(r_sb, t3, u2p, A.add)

                # chain: Rall = R0 @ R1 @ R2
                t1p = psE.tile([128, 128], F32, tag="ep")
                nc.tensor.transpose(t1p, r_sb[:, 0, :], ident)
                r0t = epool.tile([128, 128], F32, tag="r0t")
                nc.vector.tensor_copy(r0t, t1p)
                r01p = psE.tile([128, 128], F32, tag="ep")
                nc.tensor.matmul(r01p, lhsT=r0t, rhs=r_sb[:, 1, :], start=True, stop=True)
                r01 = epool.tile([128, 128], F32, tag="r01")
                nc.vector.tensor_copy(r01, r01p)
                t2p = psE.tile([128, 128], F32, tag="ep")
                nc.tensor.transpose(t2p, r01, ident)
                r01t = epool.tile([128, 128], F32, tag="r01t")
                nc.vector.tensor_copy(r01t, t2p)
                rallp = psE.tile([128, 128], F32, tag="ep")
                nc.tensor.matmul(rallp, lhsT=r01t, rhs=r_sb[:, 2, :], start=True, stop=True)
                rall = rpool.tile([128, 128], F32, tag="rall")
                nc.vector.tensor_copy(rall, rallp)
                rall_tiles.append(rall)

            # Software-pipelined einsum: chunk i's transposes are emitted
            # before chunk i-1's matmuls so the PE never idles waiting for
            # the DVE psum->sbuf copy of the chunk it just transposed. The
            # first WARM chunks were already staged before the expm.
            for i in range(WARM, len(chunks) + WARM):
                if i < len(chunks):
                    stage_chunk(i)
                j = i - WARM
                b, c, xts = staged[j]
                yp = psY.tile([128, TPC, 128], F32, tag="yp")
                for t in range(TPC):
                    nc.tensor.matmul(yp[:, t, :], lhsT=xts[:, t, :],
                                     rhs=rall_tiles[b], start=True, stop=True)
                yt = ypool.tile([128, TPC, 128], F32, tag="yt")
                nc.scalar.copy(yt, yp)
                nc.sync.dma_start(out=yr[b, c], in_=yt)
    return _split_sync_waits(nc)


_CACHE = {}


def _get(name):
    if name not in _CACHE:
        _CACHE[name] = {"pool": build_pool, "params": build_params, "rot": build_rot}[
            name
        ]()
    return _CACHE[name]


def _erf(z):
    from scipy.special import erf

    return erf(z)


def kernel(x, W1, b1, W2, b2):
    x = np.ascontiguousarray(x, dtype=np.float32)
    cores = list(range(NCORES))

    # ---- L1: pooling ----
    in1 = [{"x": x[c * BPC : (c + 1) * BPC]} for c in cores]
    r1 = run_bass_kernel_spmd(_get("pool"), in1, core_ids=cores)
    pooled = np.concatenate(
        [r1.results[c]["pooledT"].T for c in cores], axis=0
    ).astype(np.float64) / float(S)                     # [B, D]

    # ---- host: tiny MLP with exact-erf gelu ----
    pre = pooled @ W1.astype(np.float64).T + b1.astype(np.float64)
    hh = 0.5 * pre * (1.0 + _erf(pre / np.sqrt(2.0)))
    hT = np.ascontiguousarray(hh.T, dtype=np.float32)   # [H, B]

    # ---- L2: params = h @ W2.T (sharded over W2 rows) ----
    W2T = np.ascontiguousarray(W2.astype(np.float32).T)  # [H, NROT*D*D]
    in2 = [
        {"w2t": np.ascontiguousarray(W2T[:, c * JPC : (c + 1) * JPC]), "ht": hT}
        for c in cores
    ]
    r2 = run_bass_kernel_spmd(_get("params"), in2, core_ids=cores)
    params = np.empty((B, NROT * D * D), dtype=np.float32)
    for c in cores:
        params[:, c * JPC : (c + 1) * JPC] = r2.results[c]["paramsT"].T
    params += b2.astype(np.float32)

    # ---- host: skew-symmetrize ----
    P = params.reshape(B, NROT, D, D).astype(np.float64)
    G = 0.5 * (P - np.swapaxes(P, 2, 3))
    gnorm = max(
        np.linalg.norm(G[b, i], 2) for b in range(B) for i in range(NROT)
    )
    Gf = np.ascontiguousarray(G, dtype=np.float32)
    nGf = np.ascontiguousarray(-G, dtype=np.float32)

    if gnorm > 1.0:
        # Taylor-12 margin exceeded (never happens for the benchmark inputs);
        # fall back to exact host expm + device einsum-only path.
        return _fallback_host_expm(x, G)

    # ---- L3: expm + chain + einsum ----
    in3 = [
        {
            "x": x[c * BPC : (c + 1) * BPC],
            "g": Gf[c * BPC : (c + 1) * BPC],
            "ng": nGf[c * BPC : (c + 1) * BPC],
        }
        for c in cores
    ]
    r3 = run_bass_kernel_spmd(_get("rot"), in3, core_ids=cores)
    out = np.concatenate([r3.results[c]["y"] for c in cores], axis=0)
    return out


def _fallback_host_expm(x, G):
    from scipy.linalg import expm as _expm

    Rall = np.empty((B, D, D), dtype=np.float64)
    for b in range(B):
        R = np.eye(D)
        for i in range(NROT):
            R = R @ _expm(G[b, i])
        Rall[b] = R
    out = np.einsum("bnd,bde->bne", x.astype(np.float64), Rall)
    return out.astype(np.float32)
